# revision 15
# baseline (speedup 1.0000x reference)
"""Trainium2 Bass kernel for nn_Encoder (6-layer causal transformer encoder).

Sharding: 8 cores = 4 batch elements x 2-core tensor-parallel pairs.
Within a pair: attention is head-split (4 of 8 heads per core), FFN/LN/residual
are token-split (1024 of 2048 tokens per core).  Rank asymmetry is expressed
purely through ReduceScatter / AllGather rank order, so the SPMD program is
identical on every core.

v2 layout/scheduling notes:
- Activations kept feature-major ("xT": [D on partitions, T free]).
- Collectives are chunked in halves and interleaved with compute: the x
  AllGather for layer l+1 is split into AG-a (global token groups 0 and 2 -
  each rank's first local 512) and AG-b (groups 1 and 3); attention q-groups
  are processed in order 0,2,1,3 so ReduceScatter of the attention output can
  likewise fire in two halves while attention continues.
- w1/w2 are loaded whole per layer (single DMAs), no streaming.
- Partition broadcasts (softmax 1/sum, LN mean/rstd) use
  gpsimd.partition_broadcast on the otherwise idle Pool engine instead of
  DRAM bounce DMAs.
- PSUM evictions with a per-partition bias ride the scalar engine
  (activation Identity/Relu with bias); bf16 casts ride Pool.
- QK score matmuls for a head pair run concurrently via PE row tiling
  (stationary/moving at partition bases 0 and 64, contract dim 64 each).
"""

import os
import sys

sys.path.insert(0, "/opt/trn_rl_repo")

import numpy as np
import ml_dtypes

import concourse.bass as bass
import concourse.mybir as mybir
import concourse.tile as tile
from concourse import bacc, bass_utils
from concourse.masks import make_identity, make_upper_triangular

# Problem constants (hardcoded per harness contract).
B, S, V, D, F, L = 4, 2048, 32000, 512, 2048, 6
H, Dh = 8, 64
HL = H // 2            # local heads per core (4)
DL = HL * Dh           # 256 local head-dims
TOWN = S // 2          # 1024 tokens owned per core
P = 128
CC = D // P            # 4 c-chunks
FC = F // P            # 16 f-chunks
LN_EPS = 1e-5

FP32 = mybir.dt.float32
BF16 = mybir.dt.bfloat16
I32 = mybir.dt.int32

GROUPS = [[0, 1], [2, 3], [4, 5], [6, 7]]

# global token-group (512 tokens each) handled by AG/RS chunk a / b
CHUNK_TGS = {"a": (0, 2), "b": (1, 3)}

_CACHED = {}


def _build_program(no_cc=False):
    nc = bacc.Bacc("TRN2", target_bir_lowering=False, debug=False, num_devices=8)
    if no_cc:
        # benchmarking variant: collectives replaced by a local DRAM copy
        # (wrong results; identical compute/DMA structure)
        def fake_cc(kind, op, replica_groups, ins, outs, **kw):
            src = ins[0]
            dst = outs[0]
            n = min(src.size(), dst.size())
            nc.sync.dma_start(
                out=bass.AP(tensor=dst.tensor, offset=0, ap=[[1, n]]),
                in_=bass.AP(tensor=src.tensor, offset=0, ap=[[1, n]]))

        nc.gpsimd.collective_compute = fake_cc

    D_ = {}
    D_["src"] = nc.dram_tensor("src", [TOWN], I32, kind="ExternalInput")
    D_["emb"] = nc.dram_tensor("emb", [V, D], FP32, kind="ExternalInput")
    D_["wq"] = nc.dram_tensor("wq", [L, D, DL], BF16, kind="ExternalInput")
    D_["wk"] = nc.dram_tensor("wk", [L, D, DL], BF16, kind="ExternalInput")
    D_["wv"] = nc.dram_tensor("wv", [L, D, DL], BF16, kind="ExternalInput")
    D_["wo"] = nc.dram_tensor("wo", [L, DL, D], BF16, kind="ExternalInput")
    D_["bq"] = nc.dram_tensor("bq", [L, DL], FP32, kind="ExternalInput")
    D_["bk"] = nc.dram_tensor("bk", [L, DL], FP32, kind="ExternalInput")
    D_["bv"] = nc.dram_tensor("bv", [L, DL], FP32, kind="ExternalInput")
    D_["bo"] = nc.dram_tensor("bo", [L, D], FP32, kind="ExternalInput")
    D_["w1"] = nc.dram_tensor("w1", [L, D, F], BF16, kind="ExternalInput")
    D_["b1"] = nc.dram_tensor("b1", [L, F], FP32, kind="ExternalInput")
    D_["w2"] = nc.dram_tensor("w2", [L, F, D], BF16, kind="ExternalInput")
    D_["b2"] = nc.dram_tensor("b2", [L, D], FP32, kind="ExternalInput")
    D_["ln_g"] = nc.dram_tensor("ln_g", [D], FP32, kind="ExternalInput")
    D_["ln_b"] = nc.dram_tensor("ln_b", [D], FP32, kind="ExternalInput")
    D_["out"] = nc.dram_tensor("out", [TOWN, D], FP32, kind="ExternalOutput")

    # DRAM scratch, one set per layer so layers can overlap freely.
    # xh*: own normalized x chunk (AG input); xf*: gathered [2, D, 512]
    # apart*: partial O-proj (RS input); aown*: reduced own chunk (RS out)
    for c in ("a", "b"):
        D_[f"xh{c}"] = [nc.dram_tensor(f"xh{c}{l}", [D, 512], BF16, kind="Internal")
                        for l in range(L)]
        D_[f"xf{c}"] = [nc.dram_tensor(f"xf{c}{l}", [2, D, 512], BF16, kind="Internal")
                        for l in range(L)]
        D_[f"apart{c}"] = [nc.dram_tensor(f"apart{c}{l}", [2, D, 512], BF16,
                                          kind="Internal") for l in range(L)]
        D_[f"aown{c}"] = [nc.dram_tensor(f"aown{c}{l}", [D, 512], BF16,
                                         kind="Internal") for l in range(L)]

    with tile.TileContext(nc) as tc:
        _emit(nc, tc, D_)

    nc.compile()
    return nc


def _emit(nc, tc, D_):
    from contextlib import ExitStack

    ctx = ExitStack()
    Exp = mybir.ActivationFunctionType.Exp
    Relu = mybir.ActivationFunctionType.Relu
    Sqrt = mybir.ActivationFunctionType.Sqrt
    Square = mybir.ActivationFunctionType.Square
    Ident = mybir.ActivationFunctionType.Identity
    Ln = mybir.ActivationFunctionType.Ln
    ADD = mybir.AluOpType.add
    MULT = mybir.AluOpType.mult

    consts = ctx.enter_context(tc.tile_pool(name="consts", bufs=1))
    wpool = ctx.enter_context(tc.tile_pool(name="weights", bufs=1))
    wbig = ctx.enter_context(tc.tile_pool(name="wbig", bufs=1))
    acts = ctx.enter_context(tc.tile_pool(name="acts", bufs=1))
    att = ctx.enter_context(tc.tile_pool(name="att", bufs=2))
    halves = ctx.enter_context(tc.tile_pool(name="halves", bufs=2))
    small = ctx.enter_context(tc.tile_pool(name="small", bufs=2))
    expp = ctx.enter_context(tc.tile_pool(name="exp", bufs=3))
    bcast = ctx.enter_context(tc.tile_pool(name="bcast", bufs=2))
    tiny = ctx.enter_context(tc.tile_pool(name="tiny", bufs=1))
    halves1 = ctx.enter_context(tc.tile_pool(name="halves1", bufs=1))
    psA = ctx.enter_context(tc.tile_pool(name="psA", bufs=2, space="PSUM"))
    psB = ctx.enter_context(tc.tile_pool(name="psB", bufs=2, space="PSUM"))
    psC = ctx.enter_context(tc.tile_pool(name="psC", bufs=2, space="PSUM"))

    # ---- constants ----
    ident = consts.tile([P, P], FP32)
    make_identity(nc, ident)
    trimask = consts.tile([P, P], BF16)  # 1 where k<=q
    make_upper_triangular(nc, trimask, val=1.0, diag=True)
    gT = consts.tile([P, CC], FP32)
    nc.sync.dma_start(out=gT, in_=D_["ln_g"].ap().rearrange("(cc p) -> p cc", p=P))
    bT = consts.tile([P, CC], FP32)
    nc.sync.dma_start(out=bT, in_=D_["ln_b"].ap().rearrange("(cc p) -> p cc", p=P))
    ones1 = consts.tile([P, 1], BF16)
    nc.vector.memset(ones1, 1.0)
    epst = consts.tile([1, 1], FP32)
    nc.vector.memset(epst, LN_EPS)
    idx = consts.tile([P, TOWN // P], I32)
    nc.sync.dma_start(out=idx, in_=D_["src"].ap().rearrange("(tc p) -> p tc", p=P))

    # ---- long-lived activation tiles ----
    x_own = acts.tile([P, CC, TOWN], FP32, tag="x_own")      # residual stream
    xb = acts.tile([P, CC, S], BF16, tag="xb")               # gathered x
    QT = acts.tile([P, 2, S], BF16, tag="QT")
    KT = acts.tile([P, 2, S], BF16, tag="KT")
    VR = acts.tile([P, S // P, HL, Dh + 1], BF16, tag="VR")  # V rows + ones col
    nc.vector.memset(VR[:, :, :, Dh:Dh + 1], 1.0)
    y32 = acts.tile([P, CC, TOWN], FP32, tag="y32")          # LN-A output
    yb = acts.tile([P, CC, TOWN], BF16, tag="yb")            # LN-A output bf16
    h1T = acts.tile([P, FC, 512], BF16, tag="h1T")           # FFN hidden (chunk)

    # ---- layer norm on one 512-token chunk, feature-major ----
    # s: [P, CC, 512] fp32 slice view; writes out32/outb at [:, :, osl]
    def layer_norm_chunk(s, out32, outb, osl, oslb=None):
        if oslb is None:
            oslb = osl
        ps_m = psC.tile([1, 512], FP32, tag="psC", name="ps_m")
        ps_q = psC.tile([1, 512], FP32, tag="psC", name="ps_q")
        for cc in range(CC):
            chunk = small.tile([P, 512], BF16, tag="ln_chunk")
            nc.vector.tensor_copy(out=chunk, in_=s[:, cc, :])
            sqc = small.tile([P, 512], BF16, tag="ln_sqc")
            nc.scalar.activation(out=sqc, in_=s[:, cc, :], func=Square)
            nc.tensor.matmul(ps_m, ones1, chunk, start=(cc == 0), stop=(cc == CC - 1))
            nc.tensor.matmul(ps_q, ones1, sqc, start=(cc == 0), stop=(cc == CC - 1))
        mean = small.tile([1, 512], FP32, tag="ln_mean")
        nc.vector.tensor_scalar(out=mean, in0=ps_m, scalar1=1.0 / D, scalar2=0.0,
                                op0=MULT, op1=ADD)
        msq = small.tile([1, 512], FP32, tag="ln_msq")
        nc.vector.tensor_scalar(out=msq, in0=ps_q, scalar1=1.0 / D, scalar2=0.0,
                                op0=MULT, op1=ADD)  # E[x^2]
        m2 = tiny.tile([1, 512], FP32, tag="ln_m2")
        nc.vector.tensor_mul(out=m2, in0=mean, in1=mean)
        nc.vector.tensor_sub(out=msq, in0=msq, in1=m2)
        nc.scalar.activation(out=m2, in_=msq, func=Ln, bias=epst, scale=1.0)
        nc.scalar.activation(out=msq, in_=m2, func=Exp, scale=-0.5)
        rstd = msq
        mB = bcast.tile([P, 512], FP32, tag="lnB")
        nc.gpsimd.partition_broadcast(mB, mean)
        rB = bcast.tile([P, 512], FP32, tag="lnB")
        nc.gpsimd.partition_broadcast(rB, rstd)
        for cc in range(CC):
            o = out32[:, cc, osl]
            nc.vector.tensor_sub(out=o, in0=s[:, cc, :], in1=mB)
            nc.vector.tensor_mul(out=o, in0=o, in1=rB)
            nc.vector.tensor_scalar(out=o, in0=o,
                                    scalar1=gT[:, cc:cc + 1], scalar2=bT[:, cc:cc + 1],
                                    op0=MULT, op1=ADD)
            if outb is not None:
                nc.vector.tensor_copy(out=outb[:, cc, oslb], in_=o)

    # ---- QKV projection for one global token group (512 tokens) ----
    def qkv_tg(tg, wq_t, wk_t, wv_t, bq_t, bk_t, bvB):
        gsl = slice(tg * 512, (tg + 1) * 512)
        for dst, w_t, b_t in ((QT, wq_t, bq_t), (KT, wk_t, bk_t)):
            for hc in range(2):
                ps = psC.tile([P, 512], FP32, tag="psC")
                for cc in range(CC):
                    nc.tensor.matmul(ps, w_t[:, cc, hc * P:(hc + 1) * P],
                                     xb[:, cc, gsl],
                                     start=(cc == 0), stop=(cc == CC - 1))
                nc.scalar.activation(out=dst[:, hc, gsl], in_=ps, func=Ident,
                                     bias=b_t[:, hc:hc + 1])
        for tb in range(4):
            tcN = tg * 4 + tb
            ps = psC.tile([P, DL], FP32, tag="psC")
            for cc in range(CC):
                nc.tensor.matmul(ps, xb[:, cc, tcN * P:(tcN + 1) * P], wv_t[:, cc, :],
                                 start=(cc == 0), stop=(cc == CC - 1))
            nc.vector.tensor_add(
                out=VR[:, tcN, :, 0:Dh],
                in0=ps.rearrange("p (h d) -> p h d", h=HL),
                in1=bvB.rearrange("p (h d) -> p h d", h=HL))

    # ---- attention for one q-group (all local heads) + O-proj partial ----
    def attn_qg(qg, wo_t, apart_dram, shard):
        qsl = slice(qg * 512, (qg + 1) * 512)
        kmax = qg * 4 + 3
        attnT = att.tile([P, 2, 512], BF16, tag="attnT")
        for hp in range(2):
            for par in range(2):
                ho = par * Dh
                h = hp * 2 + par
                av = psB.tile([Dh + 1, 512], FP32, tag="psB", name="av")
                qt_h = QT[ho:ho + Dh, hp, qsl]
                kt_h = KT[ho:ho + Dh, hp, :]
                for kb0 in range(0, kmax + 1, 2):
                    npair = min(2, kmax + 1 - kb0)
                    sc = psA.tile([P, 1024], FP32, tag="psA")
                    for j in range(npair):
                        nc.tensor.matmul(sc[:, j * 512:(j + 1) * 512],
                                         kt_h[:, (kb0 + j) * P:(kb0 + j + 1) * P],
                                         qt_h, start=True, stop=True)
                    ex = expp.tile([P, 1024], BF16, tag="ex")
                    off0 = max(0, kb0 - qg * 4) * P
                    nc.scalar.activation(out=ex[:, off0:npair * 512],
                                         in_=sc[:, off0:npair * 512],
                                         func=Exp, scale=1.0 / 8.0)
                    for j in range(npair):
                        kb = kb0 + j
                        dj = kb - qg * 4
                        if 0 <= dj <= 3:  # diagonal block: apply causal mask
                            c0 = j * 512 + dj * P
                            nc.vector.tensor_mul(out=ex[:, c0:c0 + P],
                                                 in0=ex[:, c0:c0 + P], in1=trimask)
                        off = max(0, dj) * P
                        nc.tensor.matmul(av[:, off:], VR[:, kb, h, :],
                                         ex[:, j * 512 + off:(j + 1) * 512],
                                         start=(kb == 0), stop=(kb == kmax))
                rb0 = small.tile([1, 512], FP32, tag="rb0")
                nc.scalar.activation(out=rb0, in_=av[Dh:Dh + 1, :], func=Ln)
                nc.scalar.activation(out=rb0, in_=rb0, func=Exp, scale=-1.0)
                rb = bcast.tile([Dh, 512], FP32, tag="rb")
                nc.gpsimd.partition_broadcast(rb, rb0)
                nc.vector.tensor_mul(out=attnT[ho:ho + Dh, hp, :],
                                     in0=av[0:Dh, :], in1=rb)
        ostage = halves1.tile([P, CC, 512], BF16, tag="ostage")
        for dc in range(CC):
            ps = psC.tile([P, 512], FP32, tag="psC")
            for hc in range(2):
                nc.tensor.matmul(ps, wo_t[:, hc, dc * P:(dc + 1) * P],
                                 attnT[:, hc, :], start=(hc == 0), stop=(hc == 1))
            nc.vector.tensor_copy(out=ostage[:, dc, :], in_=ps)
        nc.sync.dma_start(
            out=apart_dram.ap()[shard].rearrange("(cc p) t -> p cc t", p=P),
            in_=ostage)

    # ---- residual-1 + LN-A + FFN + residual-2 + LN-B for one local chunk ----
    def mlp_chunk(l, c, aown_dram, bo_t, b1_t, b2_t, w1_t, w2_t):
        csl = slice(c * 512, (c + 1) * 512)
        ar = halves1.tile([P, CC, 512], BF16, tag="ar")
        nc.sync.dma_start(out=ar, in_=aown_dram.ap().rearrange("(cc p) t -> p cc t", p=P))
        s1 = halves.tile([P, CC, 512], FP32, tag="s1fo", name="s1")
        for cc in range(CC):
            nc.scalar.activation(out=s1[:, cc, :], in_=ar[:, cc, :], func=Ident,
                                 bias=bo_t[:, cc:cc + 1])
            nc.vector.tensor_add(out=s1[:, cc, :], in0=s1[:, cc, :],
                                 in1=x_own[:, cc, csl])
        layer_norm_chunk(s1, y32, yb, csl)

        # FFN
        for fc in range(FC):
            ps = psC.tile([P, 512], FP32, tag="psC")
            for cc in range(CC):
                nc.tensor.matmul(ps, w1_t[:, cc, fc * P:(fc + 1) * P], yb[:, cc, csl],
                                 start=(cc == 0), stop=(cc == CC - 1))
            nc.scalar.activation(out=h1T[:, fc, :], in_=ps, func=Relu,
                                 bias=b1_t[:, fc:fc + 1])
        fo = halves.tile([P, CC, 512], FP32, tag="s1fo", name="fo")
        for dc in range(CC):
            ps = psC.tile([P, 512], FP32, tag="psC")
            for fc in range(FC):
                nc.tensor.matmul(ps, w2_t[:, fc, dc * P:(dc + 1) * P], h1T[:, fc, :],
                                 start=(fc == 0), stop=(fc == FC - 1))
            nc.scalar.activation(out=fo[:, dc, :], in_=ps, func=Relu,
                                 bias=b2_t[:, dc:dc + 1])
            nc.vector.tensor_add(out=fo[:, dc, :], in0=fo[:, dc, :],
                                 in1=y32[:, dc, csl])

        # LN-B -> new x_own chunk (+ bf16 evict for AllGather unless last layer)
        if l < L - 1:
            xcb = tiny.tile([P, CC, 512], BF16, tag="xcb")
            layer_norm_chunk(fo, x_own, xcb, csl, slice(0, 512))
            key = "ab"[c]
            nc.sync.dma_start(
                out=D_[f"xh{key}"][l + 1].ap().rearrange("(cc p) t -> p cc t", p=P),
                in_=xcb)
            nc.gpsimd.collective_compute(
                kind="AllGather", op=mybir.AluOpType.bypass, replica_groups=GROUPS,
                ins=[D_[f"xh{key}"][l + 1].ap()], outs=[D_[f"xf{key}"][l + 1].ap()])
        else:
            layer_norm_chunk(fo, x_own, None, csl)

    # ---- load gathered-x chunk into xb (global order) ----
    def load_xb(l, key):
        xf = D_[f"xf{key}"][l]
        for s, tg in enumerate(CHUNK_TGS[key]):
            nc.sync.dma_start(
                out=xb[:, :, tg * 512:(tg + 1) * 512],
                in_=xf.ap()[s].rearrange("(cc p) t -> p cc t", p=P))

    # ---- embedding gather for own tokens -> x_own fp32 (+ AG chunks) ----
    for c in range(2):
        xcb = tiny.tile([P, CC, 512], BF16, tag="xcb")
        for tb in range(4):
            tcN = c * 4 + tb
            rows = small.tile([P, D], FP32, tag="emb_rows")
            nc.gpsimd.indirect_dma_start(
                out=rows, out_offset=None, in_=D_["emb"].ap(),
                in_offset=bass.IndirectOffsetOnAxis(ap=idx[:, tcN:tcN + 1], axis=0))
            for cc in range(CC):
                pt = psC.tile([P, 512], FP32, tag="psC", name="embT")
                nc.tensor.transpose(pt[:, 0:P], rows[:, cc * P:(cc + 1) * P], ident)
                nc.vector.tensor_copy(
                    out=x_own[:, cc, tcN * P:(tcN + 1) * P], in_=pt[:, 0:P])
                nc.scalar.copy(
                    out=xcb[:, cc, tb * P:(tb + 1) * P], in_=pt[:, 0:P])
        key = "ab"[c]
        nc.sync.dma_start(
            out=D_[f"xh{key}"][0].ap().rearrange("(cc p) t -> p cc t", p=P),
            in_=xcb)
        nc.gpsimd.collective_compute(
            kind="AllGather", op=mybir.AluOpType.bypass, replica_groups=GROUPS,
            ins=[D_[f"xh{key}"][0].ap()], outs=[D_[f"xf{key}"][0].ap()])

    for l in range(L):
        # ---- per-layer weights ----
        wq_t = wpool.tile([P, CC, DL], BF16, tag="wq")
        nc.sync.dma_start(out=wq_t, in_=D_["wq"].ap()[l].rearrange("(cc p) d -> p cc d", p=P))
        wk_t = wpool.tile([P, CC, DL], BF16, tag="wk")
        nc.sync.dma_start(out=wk_t, in_=D_["wk"].ap()[l].rearrange("(cc p) d -> p cc d", p=P))
        wv_t = wpool.tile([P, CC, DL], BF16, tag="wv")
        nc.sync.dma_start(out=wv_t, in_=D_["wv"].ap()[l].rearrange("(cc p) d -> p cc d", p=P))
        wo_t = wpool.tile([P, 2, D], BF16, tag="wo")
        nc.sync.dma_start(out=wo_t, in_=D_["wo"].ap()[l].rearrange("(hc p) d -> p hc d", p=P))
        bq_t = wpool.tile([P, 2], FP32, tag="bq")
        nc.sync.dma_start(out=bq_t, in_=D_["bq"].ap()[l].rearrange("(hc p) -> p hc", p=P))
        bk_t = wpool.tile([P, 2], FP32, tag="bk")
        nc.sync.dma_start(out=bk_t, in_=D_["bk"].ap()[l].rearrange("(hc p) -> p hc", p=P))
        bvB = wpool.tile([P, DL], FP32, tag="bvB")
        nc.sync.dma_start(out=bvB, in_=bass.AP(tensor=D_["bv"], offset=l * DL,
                                               ap=[[0, P], [1, DL]]))
        bo_t = wpool.tile([P, CC], FP32, tag="bo")
        nc.sync.dma_start(out=bo_t, in_=D_["bo"].ap()[l].rearrange("(cc p) -> p cc", p=P))
        b1_t = wpool.tile([P, FC], FP32, tag="b1")
        nc.sync.dma_start(out=b1_t, in_=D_["b1"].ap()[l].rearrange("(fc p) -> p fc", p=P))
        b2_t = wpool.tile([P, CC], FP32, tag="b2")
        nc.sync.dma_start(out=b2_t, in_=D_["b2"].ap()[l].rearrange("(cc p) -> p cc", p=P))
        w1_t = wbig.tile([P, CC, F], BF16, tag="w1")
        nc.sync.dma_start(out=w1_t, in_=D_["w1"].ap()[l].rearrange("(cc p) f -> p cc f", p=P))
        w2_t = wbig.tile([P, FC, D], BF16, tag="w2")
        nc.sync.dma_start(out=w2_t, in_=D_["w2"].ap()[l].rearrange("(fc p) d -> p fc d", p=P))

        # ---- QKV + attention, interleaved with RS chunks ----
        load_xb(l, "a")
        qkv_tg(0, wq_t, wk_t, wv_t, bq_t, bk_t, bvB)
        qkv_tg(2, wq_t, wk_t, wv_t, bq_t, bk_t, bvB)
        attn_qg(0, wo_t, D_["aparta"][l], 0)
        load_xb(l, "b")
        qkv_tg(1, wq_t, wk_t, wv_t, bq_t, bk_t, bvB)
        qkv_tg(3, wq_t, wk_t, wv_t, bq_t, bk_t, bvB)
        attn_qg(2, wo_t, D_["aparta"][l], 1)
        nc.gpsimd.collective_compute(
            kind="ReduceScatter", op=ADD, replica_groups=GROUPS,
            ins=[D_["aparta"][l].ap()], outs=[D_["aowna"][l].ap()])
        attn_qg(1, wo_t, D_["apartb"][l], 0)
        attn_qg(3, wo_t, D_["apartb"][l], 1)
        nc.gpsimd.collective_compute(
            kind="ReduceScatter", op=ADD, replica_groups=GROUPS,
            ins=[D_["apartb"][l].ap()], outs=[D_["aownb"][l].ap()])

        # ---- per-chunk residual + LN + FFN + LN (+ next-layer AG) ----
        mlp_chunk(l, 0, D_["aowna"][l], bo_t, b1_t, b2_t, w1_t, w2_t)
        mlp_chunk(l, 1, D_["aownb"][l], bo_t, b1_t, b2_t, w1_t, w2_t)

    # ---- output: transpose x_own back to rows [TOWN, D] ----
    for tb in range(TOWN // P):
        rows = small.tile([P, D], FP32, tag="emb_rows")
        for cc in range(CC):
            pt = psC.tile([P, 512], FP32, tag="psC", name="outT")
            nc.tensor.transpose(pt[:, 0:P], x_own[:, cc, tb * P:(tb + 1) * P], ident)
            nc.vector.tensor_copy(out=rows[:, cc * P:(cc + 1) * P], in_=pt[:, 0:P])
        nc.sync.dma_start(out=D_["out"].ap()[tb * P:(tb + 1) * P, :], in_=rows)

    ctx.close()


def _get_program():
    no_cc = bool(int(os.environ.get("BASS_ENC_NOCC", "0")))
    key = ("nc", no_cc)
    if key not in _CACHED:
        _CACHED[key] = _build_program(no_cc)
    return _CACHED[key]


def prep_in_maps(inputs):
    def f32(x):
        return np.ascontiguousarray(np.asarray(x, dtype=np.float32))

    def bf(x):
        return np.ascontiguousarray(np.asarray(x, dtype=np.float32).astype(ml_dtypes.bfloat16))

    source = np.asarray(inputs["source"]).astype(np.int32)
    emb = f32(inputs["emb"])
    ln_g, ln_b = f32(inputs["ln_g"]), f32(inputs["ln_b"])
    w1a, b1a = bf(inputs["w1"]), f32(inputs["b1"])
    w2a, b2a = bf(inputs["w2"]), f32(inputs["b2"])
    wqa, wka, wva = np.asarray(inputs["wq"]), np.asarray(inputs["wk"]), np.asarray(inputs["wv"])
    bqa, bka, bva = np.asarray(inputs["bq"]), np.asarray(inputs["bk"]), np.asarray(inputs["bv"])
    woa, boa = np.asarray(inputs["wo"]), f32(inputs["bo"])

    in_maps = []
    for core in range(8):
        b, half = core // 2, core % 2
        hsl = slice(half * DL, (half + 1) * DL)
        in_maps.append({
            "src": np.ascontiguousarray(source[b, half * TOWN:(half + 1) * TOWN]),
            "emb": emb,
            "wq": bf(wqa[:, :, hsl]), "wk": bf(wka[:, :, hsl]), "wv": bf(wva[:, :, hsl]),
            "bq": f32(bqa[:, hsl]), "bk": f32(bka[:, hsl]), "bv": f32(bva[:, hsl]),
            "wo": bf(woa[:, hsl, :]), "bo": boa,
            "w1": w1a, "b1": b1a, "w2": w2a, "b2": b2a,
            "ln_g": ln_g, "ln_b": ln_b,
        })
    return in_maps


def kernel(**inputs):
    nc = _get_program()
    in_maps = prep_in_maps(inputs)
    trace = bool(int(os.environ.get("BASS_ENC_TRACE", "0")))
    res = bass_utils.run_bass_kernel_spmd(nc, in_maps, core_ids=list(range(8)),
                                          trace=trace)
    _CACHED["last_results"] = res

    outp = np.empty((B, S, D), np.float32)
    for core in range(8):
        b, half = core // 2, core % 2
        outp[b, half * TOWN:(half + 1) * TOWN, :] = res.results[core]["out"]
    return outp


# revision 16
# speedup vs baseline: 1.0342x; 1.0342x over previous
"""Trainium2 Bass kernel for nn_Encoder (6-layer causal transformer encoder).

Sharding: 8 cores = 4 batch elements x 2-core tensor-parallel pairs.
Within a pair: attention is head-split (4 of 8 heads per core), FFN/LN/residual
are token-split (1024 of 2048 tokens per core).  Rank asymmetry is expressed
purely through ReduceScatter / AllGather rank order, so the SPMD program is
identical on every core.

v2 layout/scheduling notes:
- Activations kept feature-major ("xT": [D on partitions, T free]).
- Collectives are chunked in halves and interleaved with compute: the x
  AllGather for layer l+1 is split into AG-a (global token groups 0 and 2 -
  each rank's first local 512) and AG-b (groups 1 and 3); attention q-groups
  are processed in order 0,2,1,3 so ReduceScatter of the attention output can
  likewise fire in two halves while attention continues.
- w1/w2 are loaded whole per layer (single DMAs), no streaming.
- Partition broadcasts (softmax 1/sum, LN mean/rstd) use
  gpsimd.partition_broadcast on the otherwise idle Pool engine instead of
  DRAM bounce DMAs.
- PSUM evictions with a per-partition bias ride the scalar engine
  (activation Identity/Relu with bias); bf16 casts ride Pool.
- QK score matmuls for a head pair run concurrently via PE row tiling
  (stationary/moving at partition bases 0 and 64, contract dim 64 each).
"""

import os
import sys

sys.path.insert(0, "/opt/trn_rl_repo")

import numpy as np
import ml_dtypes

import concourse.bass as bass
import concourse.mybir as mybir
import concourse.tile as tile
from concourse import bacc, bass_utils
from concourse.masks import make_identity, make_upper_triangular

# Problem constants (hardcoded per harness contract).
B, S, V, D, F, L = 4, 2048, 32000, 512, 2048, 6
H, Dh = 8, 64
HL = H // 2            # local heads per core (4)
DL = HL * Dh           # 256 local head-dims
TOWN = S // 2          # 1024 tokens owned per core
P = 128
CC = D // P            # 4 c-chunks
FC = F // P            # 16 f-chunks
LN_EPS = 1e-5

FP32 = mybir.dt.float32
BF16 = mybir.dt.bfloat16
I32 = mybir.dt.int32

GROUPS = [[0, 1], [2, 3], [4, 5], [6, 7]]

# global token-group (512 tokens each) handled by AG/RS chunk a / b
CHUNK_TGS = {"a": (0, 2), "b": (1, 3)}

_CACHED = {}


def _build_program(no_cc=False):
    nc = bacc.Bacc("TRN2", target_bir_lowering=False, debug=False, num_devices=8)
    if no_cc:
        # benchmarking variant: collectives replaced by a local DRAM copy
        # (wrong results; identical compute/DMA structure)
        def fake_cc(kind, op, replica_groups, ins, outs, **kw):
            src = ins[0]
            dst = outs[0]
            n = min(src.size(), dst.size())
            nc.sync.dma_start(
                out=bass.AP(tensor=dst.tensor, offset=0, ap=[[1, n]]),
                in_=bass.AP(tensor=src.tensor, offset=0, ap=[[1, n]]))

        nc.gpsimd.collective_compute = fake_cc

    D_ = {}
    D_["src"] = nc.dram_tensor("src", [TOWN], I32, kind="ExternalInput")
    D_["emb"] = nc.dram_tensor("emb", [V, D], FP32, kind="ExternalInput")
    D_["wq"] = nc.dram_tensor("wq", [L, D, DL], BF16, kind="ExternalInput")
    D_["wk"] = nc.dram_tensor("wk", [L, D, DL], BF16, kind="ExternalInput")
    D_["wv"] = nc.dram_tensor("wv", [L, D, DL], BF16, kind="ExternalInput")
    D_["wo"] = nc.dram_tensor("wo", [L, DL, D], BF16, kind="ExternalInput")
    D_["bq"] = nc.dram_tensor("bq", [L, DL], FP32, kind="ExternalInput")
    D_["bk"] = nc.dram_tensor("bk", [L, DL], FP32, kind="ExternalInput")
    D_["bv"] = nc.dram_tensor("bv", [L, DL], FP32, kind="ExternalInput")
    D_["bo"] = nc.dram_tensor("bo", [L, D], FP32, kind="ExternalInput")
    D_["w1"] = nc.dram_tensor("w1", [L, D, F], BF16, kind="ExternalInput")
    D_["b1"] = nc.dram_tensor("b1", [L, F], FP32, kind="ExternalInput")
    D_["w2"] = nc.dram_tensor("w2", [L, F, D], BF16, kind="ExternalInput")
    D_["b2"] = nc.dram_tensor("b2", [L, D], FP32, kind="ExternalInput")
    D_["ln_g"] = nc.dram_tensor("ln_g", [D], FP32, kind="ExternalInput")
    D_["ln_b"] = nc.dram_tensor("ln_b", [D], FP32, kind="ExternalInput")
    D_["out"] = nc.dram_tensor("out", [TOWN, D], FP32, kind="ExternalOutput")

    # DRAM scratch, one set per layer so layers can overlap freely.
    # xh*: own normalized x chunk (AG input); xf*: gathered [2, D, 512]
    # apart*: partial O-proj (RS input); aown*: reduced own chunk (RS out)
    for c in ("a", "b"):
        D_[f"xh{c}"] = [nc.dram_tensor(f"xh{c}{l}", [D, 512], BF16, kind="Internal")
                        for l in range(L)]
        D_[f"xf{c}"] = [nc.dram_tensor(f"xf{c}{l}", [2, D, 512], BF16, kind="Internal")
                        for l in range(L)]
        D_[f"apart{c}"] = [nc.dram_tensor(f"apart{c}{l}", [2, D, 512], BF16,
                                          kind="Internal") for l in range(L)]
        D_[f"aown{c}"] = [nc.dram_tensor(f"aown{c}{l}", [D, 512], BF16,
                                         kind="Internal") for l in range(L)]

    with tile.TileContext(nc) as tc:
        _emit(nc, tc, D_)

    nc.compile()
    return nc


def _emit(nc, tc, D_):
    from contextlib import ExitStack

    ctx = ExitStack()
    Exp = mybir.ActivationFunctionType.Exp
    Relu = mybir.ActivationFunctionType.Relu
    Sqrt = mybir.ActivationFunctionType.Sqrt
    Square = mybir.ActivationFunctionType.Square
    Ident = mybir.ActivationFunctionType.Identity
    Ln = mybir.ActivationFunctionType.Ln
    ADD = mybir.AluOpType.add
    MULT = mybir.AluOpType.mult

    consts = ctx.enter_context(tc.tile_pool(name="consts", bufs=1))
    wpool = ctx.enter_context(tc.tile_pool(name="weights", bufs=1))
    wbig = ctx.enter_context(tc.tile_pool(name="wbig", bufs=1))
    acts = ctx.enter_context(tc.tile_pool(name="acts", bufs=1))
    att = ctx.enter_context(tc.tile_pool(name="att", bufs=2))
    halves = ctx.enter_context(tc.tile_pool(name="halves", bufs=2))
    small = ctx.enter_context(tc.tile_pool(name="small", bufs=2))
    expp = ctx.enter_context(tc.tile_pool(name="exp", bufs=3))
    bcast = ctx.enter_context(tc.tile_pool(name="bcast", bufs=2))
    tiny = ctx.enter_context(tc.tile_pool(name="tiny", bufs=1))
    halves1 = ctx.enter_context(tc.tile_pool(name="halves1", bufs=1))
    psA = ctx.enter_context(tc.tile_pool(name="psA", bufs=2, space="PSUM"))
    psB = ctx.enter_context(tc.tile_pool(name="psB", bufs=2, space="PSUM"))
    psC = ctx.enter_context(tc.tile_pool(name="psC", bufs=2, space="PSUM"))

    # ---- constants ----
    ident = consts.tile([P, P], FP32)
    make_identity(nc, ident)
    trimask = consts.tile([P, P], BF16)  # 1 where k<=q
    make_upper_triangular(nc, trimask, val=1.0, diag=True)
    gT = consts.tile([P, CC], FP32)
    nc.sync.dma_start(out=gT, in_=D_["ln_g"].ap().rearrange("(cc p) -> p cc", p=P))
    bT = consts.tile([P, CC], FP32)
    nc.sync.dma_start(out=bT, in_=D_["ln_b"].ap().rearrange("(cc p) -> p cc", p=P))
    ones1 = consts.tile([P, 1], BF16)
    nc.vector.memset(ones1, 1.0)
    epst = consts.tile([1, 1], FP32)
    nc.vector.memset(epst, LN_EPS)
    idx = consts.tile([P, TOWN // P], I32)
    nc.sync.dma_start(out=idx, in_=D_["src"].ap().rearrange("(tc p) -> p tc", p=P))

    # ---- long-lived activation tiles ----
    x_own = acts.tile([P, CC, TOWN], FP32, tag="x_own")      # residual stream
    xb = acts.tile([P, CC, S], BF16, tag="xb")               # gathered x
    QT = acts.tile([P, 2, S], BF16, tag="QT")
    KT = acts.tile([P, 2, S], BF16, tag="KT")
    VR = acts.tile([P, S // P, HL, Dh + 1], BF16, tag="VR")  # V rows + ones col
    nc.vector.memset(VR[:, :, :, Dh:Dh + 1], 1.0)
    y32 = acts.tile([P, CC, TOWN], FP32, tag="y32")          # LN-A output
    yb = acts.tile([P, CC, TOWN], BF16, tag="yb")            # LN-A output bf16
    h1T = acts.tile([P, FC, 512], BF16, tag="h1T")           # FFN hidden (chunk)

    # ---- layer norm on one 512-token chunk, feature-major ----
    # s: [P, CC, 512] fp32 slice view; writes out32/outb at [:, :, osl]
    def layer_norm_chunk(s, out32, outb, osl, oslb=None):
        if oslb is None:
            oslb = osl
        ps_m = psC.tile([1, 512], FP32, tag="psC", name="ps_m")
        ps_q = psC.tile([1, 512], FP32, tag="psC", name="ps_q")
        for cc in range(CC):
            chunk = small.tile([P, 512], BF16, tag="ln_chunk")
            nc.vector.tensor_copy(out=chunk, in_=s[:, cc, :])
            sqc = small.tile([P, 512], BF16, tag="ln_sqc")
            nc.vector.tensor_mul(out=sqc, in0=s[:, cc, :], in1=s[:, cc, :])
            nc.tensor.matmul(ps_m, ones1, chunk, start=(cc == 0), stop=(cc == CC - 1))
            nc.tensor.matmul(ps_q, ones1, sqc, start=(cc == 0), stop=(cc == CC - 1))
        mean = small.tile([1, 512], FP32, tag="ln_mean")
        nc.vector.tensor_scalar(out=mean, in0=ps_m, scalar1=1.0 / D, scalar2=0.0,
                                op0=MULT, op1=ADD)
        msq = small.tile([1, 512], FP32, tag="ln_msq")
        nc.vector.tensor_scalar(out=msq, in0=ps_q, scalar1=1.0 / D, scalar2=0.0,
                                op0=MULT, op1=ADD)  # E[x^2]
        m2 = tiny.tile([1, 512], FP32, tag="ln_m2")
        nc.vector.tensor_mul(out=m2, in0=mean, in1=mean)
        nc.vector.tensor_sub(out=msq, in0=msq, in1=m2)
        nc.scalar.activation(out=m2, in_=msq, func=Sqrt, bias=epst, scale=1.0)
        nc.vector.reciprocal(out=msq, in_=m2)
        rstd = msq
        mB = bcast.tile([P, 512], FP32, tag="lnB")
        nc.gpsimd.partition_broadcast(mB, mean)
        rB = bcast.tile([P, 512], FP32, tag="lnB")
        nc.gpsimd.partition_broadcast(rB, rstd)
        for cc in range(CC):
            o = out32[:, cc, osl]
            nc.vector.tensor_sub(out=o, in0=s[:, cc, :], in1=mB)
            nc.vector.tensor_mul(out=o, in0=o, in1=rB)
            nc.vector.tensor_scalar(out=o, in0=o,
                                    scalar1=gT[:, cc:cc + 1], scalar2=bT[:, cc:cc + 1],
                                    op0=MULT, op1=ADD)
            if outb is not None:
                nc.vector.tensor_copy(out=outb[:, cc, oslb], in_=o)

    # ---- QKV projection for one global token group (512 tokens) ----
    def qkv_tg(tg, wq_t, wk_t, wv_t, bq_t, bk_t, bvB):
        gsl = slice(tg * 512, (tg + 1) * 512)
        for dst, w_t, b_t in ((QT, wq_t, bq_t), (KT, wk_t, bk_t)):
            for hc in range(2):
                ps = psC.tile([P, 512], FP32, tag="psC")
                for cc in range(CC):
                    nc.tensor.matmul(ps, w_t[:, cc, hc * P:(hc + 1) * P],
                                     xb[:, cc, gsl],
                                     start=(cc == 0), stop=(cc == CC - 1))
                nc.vector.tensor_scalar_add(out=dst[:, hc, gsl], in0=ps,
                                            scalar1=b_t[:, hc:hc + 1])
        for tb in range(4):
            tcN = tg * 4 + tb
            ps = psC.tile([P, DL], FP32, tag="psC")
            for cc in range(CC):
                nc.tensor.matmul(ps, xb[:, cc, tcN * P:(tcN + 1) * P], wv_t[:, cc, :],
                                 start=(cc == 0), stop=(cc == CC - 1))
            nc.vector.tensor_add(
                out=VR[:, tcN, :, 0:Dh],
                in0=ps.rearrange("p (h d) -> p h d", h=HL),
                in1=bvB.rearrange("p (h d) -> p h d", h=HL))

    # ---- attention for one q-group (all local heads) + O-proj partial ----
    def attn_qg(qg, wo_t, apart_dram, shard):
        qsl = slice(qg * 512, (qg + 1) * 512)
        kmax = qg * 4 + 3
        attnT = att.tile([P, 2, 512], BF16, tag="attnT")
        for hp in range(2):
            for par in range(2):
                ho = par * Dh
                h = hp * 2 + par
                av = psB.tile([Dh + 1, 512], FP32, tag="psB", name="av")
                qt_h = QT[ho:ho + Dh, hp, qsl]
                kt_h = KT[ho:ho + Dh, hp, :]
                for kb0 in range(0, kmax + 1, 2):
                    npair = min(2, kmax + 1 - kb0)
                    sc = psA.tile([P, 1024], FP32, tag="psA")
                    for j in range(npair):
                        nc.tensor.matmul(sc[:, j * 512:(j + 1) * 512],
                                         kt_h[:, (kb0 + j) * P:(kb0 + j + 1) * P],
                                         qt_h, start=True, stop=True)
                    ex = expp.tile([P, 1024], BF16, tag="ex")
                    off0 = max(0, kb0 - qg * 4) * P
                    nc.scalar.activation(out=ex[:, off0:npair * 512],
                                         in_=sc[:, off0:npair * 512],
                                         func=Exp, scale=1.0 / 8.0)
                    for j in range(npair):
                        kb = kb0 + j
                        dj = kb - qg * 4
                        if 0 <= dj <= 3:  # diagonal block: apply causal mask
                            c0 = j * 512 + dj * P
                            nc.vector.tensor_mul(out=ex[:, c0:c0 + P],
                                                 in0=ex[:, c0:c0 + P], in1=trimask)
                        off = max(0, dj) * P
                        nc.tensor.matmul(av[:, off:], VR[:, kb, h, :],
                                         ex[:, j * 512 + off:(j + 1) * 512],
                                         start=(kb == 0), stop=(kb == kmax))
                rb0 = small.tile([1, 512], FP32, tag="rb0")
                nc.vector.reciprocal(out=rb0, in_=av[Dh:Dh + 1, :])
                rb = bcast.tile([Dh, 512], FP32, tag="rb")
                nc.gpsimd.partition_broadcast(rb, rb0)
                nc.vector.tensor_mul(out=attnT[ho:ho + Dh, hp, :],
                                     in0=av[0:Dh, :], in1=rb)
        ostage = halves1.tile([P, CC, 512], BF16, tag="ostage")
        for dc in range(CC):
            ps = psC.tile([P, 512], FP32, tag="psC")
            for hc in range(2):
                nc.tensor.matmul(ps, wo_t[:, hc, dc * P:(dc + 1) * P],
                                 attnT[:, hc, :], start=(hc == 0), stop=(hc == 1))
            nc.vector.tensor_copy(out=ostage[:, dc, :], in_=ps)
        nc.sync.dma_start(
            out=apart_dram.ap()[shard].rearrange("(cc p) t -> p cc t", p=P),
            in_=ostage)

    # ---- residual-1 + LN-A + FFN + residual-2 + LN-B for one local chunk ----
    def mlp_chunk(l, c, aown_dram, bo_t, b1_t, b2_t, w1_t, w2_t):
        csl = slice(c * 512, (c + 1) * 512)
        ar = halves1.tile([P, CC, 512], BF16, tag="ar")
        nc.sync.dma_start(out=ar, in_=aown_dram.ap().rearrange("(cc p) t -> p cc t", p=P))
        s1 = halves.tile([P, CC, 512], FP32, tag="s1fo", name="s1")
        for cc in range(CC):
            nc.vector.tensor_scalar_add(out=s1[:, cc, :], in0=ar[:, cc, :],
                                        scalar1=bo_t[:, cc:cc + 1])
            nc.vector.tensor_add(out=s1[:, cc, :], in0=s1[:, cc, :],
                                 in1=x_own[:, cc, csl])
        layer_norm_chunk(s1, y32, yb, csl)

        # FFN
        for fc in range(FC):
            ps = psC.tile([P, 512], FP32, tag="psC")
            for cc in range(CC):
                nc.tensor.matmul(ps, w1_t[:, cc, fc * P:(fc + 1) * P], yb[:, cc, csl],
                                 start=(cc == 0), stop=(cc == CC - 1))
            nc.scalar.activation(out=h1T[:, fc, :], in_=ps, func=Relu,
                                 bias=b1_t[:, fc:fc + 1])
        fo = halves.tile([P, CC, 512], FP32, tag="s1fo", name="fo")
        for dc in range(CC):
            ps = psC.tile([P, 512], FP32, tag="psC")
            for fc in range(FC):
                nc.tensor.matmul(ps, w2_t[:, fc, dc * P:(dc + 1) * P], h1T[:, fc, :],
                                 start=(fc == 0), stop=(fc == FC - 1))
            nc.scalar.activation(out=fo[:, dc, :], in_=ps, func=Relu,
                                 bias=b2_t[:, dc:dc + 1])
            nc.vector.tensor_add(out=fo[:, dc, :], in0=fo[:, dc, :],
                                 in1=y32[:, dc, csl])

        # LN-B -> new x_own chunk (+ bf16 evict for AllGather unless last layer)
        if l < L - 1:
            xcb = tiny.tile([P, CC, 512], BF16, tag="xcb")
            layer_norm_chunk(fo, x_own, xcb, csl, slice(0, 512))
            key = "ab"[c]
            nc.sync.dma_start(
                out=D_[f"xh{key}"][l + 1].ap().rearrange("(cc p) t -> p cc t", p=P),
                in_=xcb)
            nc.gpsimd.collective_compute(
                kind="AllGather", op=mybir.AluOpType.bypass, replica_groups=GROUPS,
                ins=[D_[f"xh{key}"][l + 1].ap()], outs=[D_[f"xf{key}"][l + 1].ap()])
        else:
            layer_norm_chunk(fo, x_own, None, csl)

    # ---- load gathered-x chunk into xb (global order) ----
    def load_xb(l, key):
        xf = D_[f"xf{key}"][l]
        for s, tg in enumerate(CHUNK_TGS[key]):
            nc.sync.dma_start(
                out=xb[:, :, tg * 512:(tg + 1) * 512],
                in_=xf.ap()[s].rearrange("(cc p) t -> p cc t", p=P))

    # ---- embedding gather for own tokens -> x_own fp32 (+ AG chunks) ----
    for c in range(2):
        xcb = tiny.tile([P, CC, 512], BF16, tag="xcb")
        for tb in range(4):
            tcN = c * 4 + tb
            rows = small.tile([P, D], FP32, tag="emb_rows")
            nc.gpsimd.indirect_dma_start(
                out=rows, out_offset=None, in_=D_["emb"].ap(),
                in_offset=bass.IndirectOffsetOnAxis(ap=idx[:, tcN:tcN + 1], axis=0))
            for cc in range(CC):
                pt = psC.tile([P, 512], FP32, tag="psC", name="embT")
                nc.tensor.transpose(pt[:, 0:P], rows[:, cc * P:(cc + 1) * P], ident)
                nc.vector.tensor_copy(
                    out=x_own[:, cc, tcN * P:(tcN + 1) * P], in_=pt[:, 0:P])
                nc.scalar.copy(
                    out=xcb[:, cc, tb * P:(tb + 1) * P], in_=pt[:, 0:P])
        key = "ab"[c]
        nc.sync.dma_start(
            out=D_[f"xh{key}"][0].ap().rearrange("(cc p) t -> p cc t", p=P),
            in_=xcb)
        nc.gpsimd.collective_compute(
            kind="AllGather", op=mybir.AluOpType.bypass, replica_groups=GROUPS,
            ins=[D_[f"xh{key}"][0].ap()], outs=[D_[f"xf{key}"][0].ap()])

    for l in range(L):
        # ---- per-layer weights ----
        wq_t = wpool.tile([P, CC, DL], BF16, tag="wq")
        nc.sync.dma_start(out=wq_t, in_=D_["wq"].ap()[l].rearrange("(cc p) d -> p cc d", p=P))
        wk_t = wpool.tile([P, CC, DL], BF16, tag="wk")
        nc.sync.dma_start(out=wk_t, in_=D_["wk"].ap()[l].rearrange("(cc p) d -> p cc d", p=P))
        wv_t = wpool.tile([P, CC, DL], BF16, tag="wv")
        nc.sync.dma_start(out=wv_t, in_=D_["wv"].ap()[l].rearrange("(cc p) d -> p cc d", p=P))
        wo_t = wpool.tile([P, 2, D], BF16, tag="wo")
        nc.sync.dma_start(out=wo_t, in_=D_["wo"].ap()[l].rearrange("(hc p) d -> p hc d", p=P))
        bq_t = wpool.tile([P, 2], FP32, tag="bq")
        nc.sync.dma_start(out=bq_t, in_=D_["bq"].ap()[l].rearrange("(hc p) -> p hc", p=P))
        bk_t = wpool.tile([P, 2], FP32, tag="bk")
        nc.sync.dma_start(out=bk_t, in_=D_["bk"].ap()[l].rearrange("(hc p) -> p hc", p=P))
        bvB = wpool.tile([P, DL], FP32, tag="bvB")
        nc.sync.dma_start(out=bvB, in_=bass.AP(tensor=D_["bv"], offset=l * DL,
                                               ap=[[0, P], [1, DL]]))
        bo_t = wpool.tile([P, CC], FP32, tag="bo")
        nc.sync.dma_start(out=bo_t, in_=D_["bo"].ap()[l].rearrange("(cc p) -> p cc", p=P))
        b1_t = wpool.tile([P, FC], FP32, tag="b1")
        nc.sync.dma_start(out=b1_t, in_=D_["b1"].ap()[l].rearrange("(fc p) -> p fc", p=P))
        b2_t = wpool.tile([P, CC], FP32, tag="b2")
        nc.sync.dma_start(out=b2_t, in_=D_["b2"].ap()[l].rearrange("(cc p) -> p cc", p=P))
        w1_t = wbig.tile([P, CC, F], BF16, tag="w1")
        nc.sync.dma_start(out=w1_t, in_=D_["w1"].ap()[l].rearrange("(cc p) f -> p cc f", p=P))
        w2_t = wbig.tile([P, FC, D], BF16, tag="w2")
        nc.sync.dma_start(out=w2_t, in_=D_["w2"].ap()[l].rearrange("(fc p) d -> p fc d", p=P))

        # ---- QKV + attention, interleaved with RS chunks ----
        load_xb(l, "a")
        qkv_tg(0, wq_t, wk_t, wv_t, bq_t, bk_t, bvB)
        qkv_tg(2, wq_t, wk_t, wv_t, bq_t, bk_t, bvB)
        attn_qg(0, wo_t, D_["aparta"][l], 0)
        load_xb(l, "b")
        qkv_tg(1, wq_t, wk_t, wv_t, bq_t, bk_t, bvB)
        qkv_tg(3, wq_t, wk_t, wv_t, bq_t, bk_t, bvB)
        attn_qg(2, wo_t, D_["aparta"][l], 1)
        nc.gpsimd.collective_compute(
            kind="ReduceScatter", op=ADD, replica_groups=GROUPS,
            ins=[D_["aparta"][l].ap()], outs=[D_["aowna"][l].ap()])
        attn_qg(1, wo_t, D_["apartb"][l], 0)
        attn_qg(3, wo_t, D_["apartb"][l], 1)
        nc.gpsimd.collective_compute(
            kind="ReduceScatter", op=ADD, replica_groups=GROUPS,
            ins=[D_["apartb"][l].ap()], outs=[D_["aownb"][l].ap()])

        # ---- per-chunk residual + LN + FFN + LN (+ next-layer AG) ----
        mlp_chunk(l, 0, D_["aowna"][l], bo_t, b1_t, b2_t, w1_t, w2_t)
        mlp_chunk(l, 1, D_["aownb"][l], bo_t, b1_t, b2_t, w1_t, w2_t)

    # ---- output: transpose x_own back to rows [TOWN, D] ----
    for tb in range(TOWN // P):
        rows = small.tile([P, D], FP32, tag="emb_rows")
        for cc in range(CC):
            pt = psC.tile([P, 512], FP32, tag="psC", name="outT")
            nc.tensor.transpose(pt[:, 0:P], x_own[:, cc, tb * P:(tb + 1) * P], ident)
            nc.vector.tensor_copy(out=rows[:, cc * P:(cc + 1) * P], in_=pt[:, 0:P])
        nc.sync.dma_start(out=D_["out"].ap()[tb * P:(tb + 1) * P, :], in_=rows)

    ctx.close()


def _get_program():
    no_cc = bool(int(os.environ.get("BASS_ENC_NOCC", "0")))
    key = ("nc", no_cc)
    if key not in _CACHED:
        _CACHED[key] = _build_program(no_cc)
    return _CACHED[key]


def prep_in_maps(inputs):
    def f32(x):
        return np.ascontiguousarray(np.asarray(x, dtype=np.float32))

    def bf(x):
        return np.ascontiguousarray(np.asarray(x, dtype=np.float32).astype(ml_dtypes.bfloat16))

    source = np.asarray(inputs["source"]).astype(np.int32)
    emb = f32(inputs["emb"])
    ln_g, ln_b = f32(inputs["ln_g"]), f32(inputs["ln_b"])
    w1a, b1a = bf(inputs["w1"]), f32(inputs["b1"])
    w2a, b2a = bf(inputs["w2"]), f32(inputs["b2"])
    wqa, wka, wva = np.asarray(inputs["wq"]), np.asarray(inputs["wk"]), np.asarray(inputs["wv"])
    bqa, bka, bva = np.asarray(inputs["bq"]), np.asarray(inputs["bk"]), np.asarray(inputs["bv"])
    woa, boa = np.asarray(inputs["wo"]), f32(inputs["bo"])

    in_maps = []
    for core in range(8):
        b, half = core // 2, core % 2
        hsl = slice(half * DL, (half + 1) * DL)
        in_maps.append({
            "src": np.ascontiguousarray(source[b, half * TOWN:(half + 1) * TOWN]),
            "emb": emb,
            "wq": bf(wqa[:, :, hsl]), "wk": bf(wka[:, :, hsl]), "wv": bf(wva[:, :, hsl]),
            "bq": f32(bqa[:, hsl]), "bk": f32(bka[:, hsl]), "bv": f32(bva[:, hsl]),
            "wo": bf(woa[:, hsl, :]), "bo": boa,
            "w1": w1a, "b1": b1a, "w2": w2a, "b2": b2a,
            "ln_g": ln_g, "ln_b": ln_b,
        })
    return in_maps


def kernel(**inputs):
    nc = _get_program()
    in_maps = prep_in_maps(inputs)
    trace = bool(int(os.environ.get("BASS_ENC_TRACE", "0")))
    res = bass_utils.run_bass_kernel_spmd(nc, in_maps, core_ids=list(range(8)),
                                          trace=trace)
    _CACHED["last_results"] = res

    outp = np.empty((B, S, D), np.float32)
    for core in range(8):
        b, half = core // 2, core % 2
        outp[b, half * TOWN:(half + 1) * TOWN, :] = res.results[core]["out"]
    return outp


# revision 18
# speedup vs baseline: 1.2716x; 1.2295x over previous
"""Trainium2 Bass kernel for nn_Encoder (6-layer causal transformer encoder).

Sharding: 8 cores = 4 batch elements x 2-core tensor-parallel pairs.
Within a pair: attention is head-split (4 of 8 heads per core), FFN/LN/residual
are token-split (1024 of 2048 tokens per core).  Rank asymmetry is expressed
purely through ReduceScatter / AllGather rank order, so the SPMD program is
identical on every core.

v2 layout/scheduling notes:
- Activations kept feature-major ("xT": [D on partitions, T free]).
- Collectives are chunked in halves and interleaved with compute: the x
  AllGather for layer l+1 is split into AG-a (global token groups 0 and 2 -
  each rank's first local 512) and AG-b (groups 1 and 3); attention q-groups
  are processed in order 0,2,1,3 so ReduceScatter of the attention output can
  likewise fire in two halves while attention continues.
- w1/w2 are loaded whole per layer (single DMAs), no streaming.
- Partition broadcasts (softmax 1/sum, LN mean/rstd) use
  gpsimd.partition_broadcast on the otherwise idle Pool engine instead of
  DRAM bounce DMAs.
- PSUM evictions with a per-partition bias ride the scalar engine
  (activation Identity/Relu with bias); bf16 casts ride Pool.
- QK score matmuls for a head pair run concurrently via PE row tiling
  (stationary/moving at partition bases 0 and 64, contract dim 64 each).
"""

import os
import sys

sys.path.insert(0, "/opt/trn_rl_repo")

import numpy as np
import ml_dtypes

import concourse.bass as bass
import concourse.mybir as mybir
import concourse.tile as tile
from concourse import bacc, bass_utils
from concourse.masks import make_identity, make_upper_triangular

# Problem constants (hardcoded per harness contract).
B, S, V, D, F, L = 4, 2048, 32000, 512, 2048, 6
H, Dh = 8, 64
HL = H // 2            # local heads per core (4)
DL = HL * Dh           # 256 local head-dims
TOWN = S // 2          # 1024 tokens owned per core
P = 128
CC = D // P            # 4 c-chunks
FC = F // P            # 16 f-chunks
LN_EPS = 1e-5

FP32 = mybir.dt.float32
BF16 = mybir.dt.bfloat16
I32 = mybir.dt.int32

GROUPS = [[0, 1], [2, 3], [4, 5], [6, 7]]

# global token-group (512 tokens each) handled by AG/RS chunk a / b
CHUNK_TGS = {"a": (0, 2), "b": (1, 3)}

_CACHED = {}


def _build_program(no_cc=False):
    nc = bacc.Bacc("TRN2", target_bir_lowering=False, debug=False, num_devices=8)
    if no_cc:
        # benchmarking variant: collectives replaced by a local DRAM copy
        # (wrong results; identical compute/DMA structure)
        def fake_cc(kind, op, replica_groups, ins, outs, **kw):
            src = ins[0]
            dst = outs[0]
            n = min(src.size(), dst.size())
            nc.sync.dma_start(
                out=bass.AP(tensor=dst.tensor, offset=0, ap=[[1, n]]),
                in_=bass.AP(tensor=src.tensor, offset=0, ap=[[1, n]]))

        nc.gpsimd.collective_compute = fake_cc

    D_ = {}
    D_["src"] = nc.dram_tensor("src", [TOWN], I32, kind="ExternalInput")
    D_["emb"] = nc.dram_tensor("emb", [V, D], FP32, kind="ExternalInput")
    D_["wq"] = nc.dram_tensor("wq", [L, D, DL], BF16, kind="ExternalInput")
    D_["wk"] = nc.dram_tensor("wk", [L, D, DL], BF16, kind="ExternalInput")
    D_["wv"] = nc.dram_tensor("wv", [L, D, DL], BF16, kind="ExternalInput")
    D_["wo"] = nc.dram_tensor("wo", [L, DL, D], BF16, kind="ExternalInput")
    D_["bq"] = nc.dram_tensor("bq", [L, DL], FP32, kind="ExternalInput")
    D_["bk"] = nc.dram_tensor("bk", [L, DL], FP32, kind="ExternalInput")
    D_["bv"] = nc.dram_tensor("bv", [L, DL], FP32, kind="ExternalInput")
    D_["bo"] = nc.dram_tensor("bo", [L, D], FP32, kind="ExternalInput")
    D_["w1"] = nc.dram_tensor("w1", [L, D, F], BF16, kind="ExternalInput")
    D_["b1"] = nc.dram_tensor("b1", [L, F], FP32, kind="ExternalInput")
    D_["w2"] = nc.dram_tensor("w2", [L, F, D], BF16, kind="ExternalInput")
    D_["b2"] = nc.dram_tensor("b2", [L, D], FP32, kind="ExternalInput")
    D_["ln_g"] = nc.dram_tensor("ln_g", [D], FP32, kind="ExternalInput")
    D_["ln_b"] = nc.dram_tensor("ln_b", [D], FP32, kind="ExternalInput")
    D_["out"] = nc.dram_tensor("out", [TOWN, D], FP32, kind="ExternalOutput")

    # DRAM scratch, one set per layer so layers can overlap freely.
    # xh*: own normalized x chunk (AG input); xf*: gathered [2, D, 512]
    # apart*: partial O-proj (RS input); aown*: reduced own chunk (RS out)
    for c in ("a", "b"):
        D_[f"xh{c}"] = [nc.dram_tensor(f"xh{c}{l}", [D, 512], BF16, kind="Internal")
                        for l in range(L)]
        D_[f"xf{c}"] = [nc.dram_tensor(f"xf{c}{l}", [2, D, 512], BF16, kind="Internal")
                        for l in range(L)]
        D_[f"apart{c}"] = [nc.dram_tensor(f"apart{c}{l}", [2, D, 512], BF16,
                                          kind="Internal") for l in range(L)]
        D_[f"aown{c}"] = [nc.dram_tensor(f"aown{c}{l}", [D, 512], BF16,
                                         kind="Internal") for l in range(L)]

    with tile.TileContext(nc) as tc:
        _emit(nc, tc, D_)

    nc.compile()
    return nc


def _emit(nc, tc, D_):
    from contextlib import ExitStack

    ctx = ExitStack()
    Exp = mybir.ActivationFunctionType.Exp
    Relu = mybir.ActivationFunctionType.Relu
    Sqrt = mybir.ActivationFunctionType.Sqrt
    Square = mybir.ActivationFunctionType.Square
    Ident = mybir.ActivationFunctionType.Identity
    Ln = mybir.ActivationFunctionType.Ln
    ADD = mybir.AluOpType.add
    MULT = mybir.AluOpType.mult

    consts = ctx.enter_context(tc.tile_pool(name="consts", bufs=1))
    wpool = ctx.enter_context(tc.tile_pool(name="weights", bufs=1))
    wbig = ctx.enter_context(tc.tile_pool(name="wbig", bufs=1))
    acts = ctx.enter_context(tc.tile_pool(name="acts", bufs=1))
    att = ctx.enter_context(tc.tile_pool(name="att", bufs=2))
    halves = ctx.enter_context(tc.tile_pool(name="halves", bufs=2))
    small = ctx.enter_context(tc.tile_pool(name="small", bufs=2))
    expp = ctx.enter_context(tc.tile_pool(name="exp", bufs=3))
    bcast = ctx.enter_context(tc.tile_pool(name="bcast", bufs=2))
    tiny = ctx.enter_context(tc.tile_pool(name="tiny", bufs=1))
    halves1 = ctx.enter_context(tc.tile_pool(name="halves1", bufs=1))
    psA = ctx.enter_context(tc.tile_pool(name="psA", bufs=2, space="PSUM"))
    psB = ctx.enter_context(tc.tile_pool(name="psB", bufs=2, space="PSUM"))
    psC = ctx.enter_context(tc.tile_pool(name="psC", bufs=2, space="PSUM"))

    # ---- constants ----
    ident = consts.tile([P, P], FP32)
    make_identity(nc, ident)
    trimask = consts.tile([P, P], BF16)  # 1 where k<=q
    make_upper_triangular(nc, trimask, val=1.0, diag=True)
    gT = consts.tile([P, CC], FP32)
    nc.sync.dma_start(out=gT, in_=D_["ln_g"].ap().rearrange("(cc p) -> p cc", p=P))
    bT = consts.tile([P, CC], FP32)
    nc.sync.dma_start(out=bT, in_=D_["ln_b"].ap().rearrange("(cc p) -> p cc", p=P))
    ones1 = consts.tile([P, 1], BF16)
    nc.vector.memset(ones1, 1.0)
    epst = consts.tile([1, 1], FP32)
    nc.vector.memset(epst, LN_EPS)
    idx = consts.tile([P, TOWN // P], I32)
    nc.sync.dma_start(out=idx, in_=D_["src"].ap().rearrange("(tc p) -> p tc", p=P))

    # ---- long-lived activation tiles ----
    x_own = acts.tile([P, CC, TOWN], FP32, tag="x_own")      # residual stream
    xb = acts.tile([P, CC, S], BF16, tag="xb")               # gathered x
    QT = acts.tile([P, 2, S], BF16, tag="QT")
    KT = acts.tile([P, 2, S], BF16, tag="KT")
    VR = acts.tile([P, S // P, HL, Dh + 1], BF16, tag="VR")  # V rows + ones col
    nc.vector.memset(VR[:, :, :, Dh:Dh + 1], 1.0)
    y32 = acts.tile([P, CC, TOWN], FP32, tag="y32")          # LN-A output
    yb = acts.tile([P, CC, TOWN], BF16, tag="yb")            # LN-A output bf16
    h1T = acts.tile([P, FC, 512], BF16, tag="h1T")           # FFN hidden (chunk)

    # ---- layer norm on one 512-token chunk, feature-major ----
    # s: [P, CC, 512] fp32 slice view; writes out32/outb at [:, :, osl]
    def layer_norm_chunk(s, out32, outb, osl, oslb=None):
        if oslb is None:
            oslb = osl
        ps_m = psC.tile([1, 512], FP32, tag="psC", name="ps_m")
        ps_q = psC.tile([1, 512], FP32, tag="psC", name="ps_q")
        for cc in range(CC):
            chunk = small.tile([P, 512], BF16, tag="ln_chunk")
            nc.vector.tensor_copy(out=chunk, in_=s[:, cc, :])
            sqc = small.tile([P, 512], BF16, tag="ln_sqc")
            nc.vector.tensor_mul(out=sqc, in0=s[:, cc, :], in1=s[:, cc, :])
            nc.tensor.matmul(ps_m, ones1, chunk, start=(cc == 0), stop=(cc == CC - 1))
            nc.tensor.matmul(ps_q, ones1, sqc, start=(cc == 0), stop=(cc == CC - 1))
        mean = small.tile([1, 512], FP32, tag="ln_mean")
        nc.vector.tensor_scalar(out=mean, in0=ps_m, scalar1=1.0 / D, scalar2=0.0,
                                op0=MULT, op1=ADD)
        msq = small.tile([1, 512], FP32, tag="ln_msq")
        nc.vector.tensor_scalar(out=msq, in0=ps_q, scalar1=1.0 / D, scalar2=0.0,
                                op0=MULT, op1=ADD)  # E[x^2]
        m2 = tiny.tile([1, 512], FP32, tag="ln_m2")
        nc.vector.tensor_mul(out=m2, in0=mean, in1=mean)
        nc.vector.tensor_sub(out=msq, in0=msq, in1=m2)
        nc.scalar.activation(out=m2, in_=msq, func=Sqrt, bias=epst, scale=1.0)
        nc.vector.reciprocal(out=msq, in_=m2)
        rstd = msq
        mB = bcast.tile([P, 512], FP32, tag="lnB")
        nc.gpsimd.partition_broadcast(mB, mean)
        rB = bcast.tile([P, 512], FP32, tag="lnB")
        nc.gpsimd.partition_broadcast(rB, rstd)
        for cc in range(CC):
            o = out32[:, cc, osl]
            nc.vector.tensor_sub(out=o, in0=s[:, cc, :], in1=mB)
            nc.vector.tensor_mul(out=o, in0=o, in1=rB)
            nc.vector.tensor_scalar(out=o, in0=o,
                                    scalar1=gT[:, cc:cc + 1], scalar2=bT[:, cc:cc + 1],
                                    op0=MULT, op1=ADD)
            if outb is not None:
                nc.vector.tensor_copy(out=outb[:, cc, oslb], in_=o)

    # ---- QKV projection for one global token group (512 tokens) ----
    def qkv_tg(tg, wq_t, wk_t, wv_t, bq_t, bk_t, bvB):
        gsl = slice(tg * 512, (tg + 1) * 512)
        for dst, w_t, b_t in ((QT, wq_t, bq_t), (KT, wk_t, bk_t)):
            for hc in range(2):
                ps = psC.tile([P, 512], FP32, tag="psC")
                for cc in range(CC):
                    nc.tensor.matmul(ps, w_t[:, cc, hc * P:(hc + 1) * P],
                                     xb[:, cc, gsl],
                                     start=(cc == 0), stop=(cc == CC - 1))
                nc.vector.tensor_scalar_add(out=dst[:, hc, gsl], in0=ps,
                                            scalar1=b_t[:, hc:hc + 1])
        for tb in range(4):
            tcN = tg * 4 + tb
            ps = psC.tile([P, DL], FP32, tag="psC")
            for cc in range(CC):
                nc.tensor.matmul(ps, xb[:, cc, tcN * P:(tcN + 1) * P], wv_t[:, cc, :],
                                 start=(cc == 0), stop=(cc == CC - 1))
            nc.vector.tensor_add(
                out=VR[:, tcN, :, 0:Dh],
                in0=ps.rearrange("p (h d) -> p h d", h=HL),
                in1=bvB.rearrange("p (h d) -> p h d", h=HL))

    # ---- attention for one q-group (all local heads) + O-proj partial ----
    def attn_qg(qg, wo_t, apart_dram, shard):
        qsl = slice(qg * 512, (qg + 1) * 512)
        kmax = qg * 4 + 3
        attnT = att.tile([P, 2, 512], BF16, tag="attnT")
        for hp in range(2):
            for par in range(2):
                ho = par * Dh
                h = hp * 2 + par
                av = psB.tile([Dh + 1, 512], FP32, tag="psB", name="av")
                qt_h = QT[ho:ho + Dh, hp, qsl]
                kt_h = KT[ho:ho + Dh, hp, :]
                for kb0 in range(0, kmax + 1, 2):
                    npair = min(2, kmax + 1 - kb0)
                    sc = psA.tile([P, 1024], FP32, tag="psA")
                    for j in range(npair):
                        nc.tensor.matmul(sc[:, j * 512:(j + 1) * 512],
                                         kt_h[:, (kb0 + j) * P:(kb0 + j + 1) * P],
                                         qt_h, start=True, stop=True)
                    ex = expp.tile([P, 1024], BF16, tag="ex")
                    off0 = max(0, kb0 - qg * 4) * P
                    nc.scalar.activation(out=ex[:, off0:npair * 512],
                                         in_=sc[:, off0:npair * 512],
                                         func=Exp, scale=1.0 / 8.0)
                    for j in range(npair):
                        kb = kb0 + j
                        dj = kb - qg * 4
                        if 0 <= dj <= 3:  # diagonal block: apply causal mask
                            c0 = j * 512 + dj * P
                            nc.vector.tensor_mul(out=ex[:, c0:c0 + P],
                                                 in0=ex[:, c0:c0 + P], in1=trimask)
                        off = max(0, dj) * P
                        nc.tensor.matmul(av[:, off:], VR[:, kb, h, :],
                                         ex[:, j * 512 + off:(j + 1) * 512],
                                         start=(kb == 0), stop=(kb == kmax))
                rb0 = small.tile([1, 512], FP32, tag="rb0")
                nc.vector.reciprocal(out=rb0, in_=av[Dh:Dh + 1, :])
                rb = bcast.tile([Dh, 512], FP32, tag="rb")
                nc.gpsimd.partition_broadcast(rb, rb0)
                nc.vector.tensor_mul(out=attnT[ho:ho + Dh, hp, :],
                                     in0=av[0:Dh, :], in1=rb)
        ostage = halves1.tile([P, CC, 512], BF16, tag="ostage")
        for dc in range(CC):
            ps = psC.tile([P, 512], FP32, tag="psC")
            for hc in range(2):
                nc.tensor.matmul(ps, wo_t[:, hc, dc * P:(dc + 1) * P],
                                 attnT[:, hc, :], start=(hc == 0), stop=(hc == 1))
            nc.vector.tensor_copy(out=ostage[:, dc, :], in_=ps)
        nc.sync.dma_start(
            out=apart_dram.ap()[shard].rearrange("(cc p) t -> p cc t", p=P),
            in_=ostage)

    # ---- residual-1 + LN-A + FFN + residual-2 + LN-B for one local chunk ----
    def mlp_chunk(l, c, aown_dram, bo_t, b1_t, b2_t, w1_t, w2_t):
        csl = slice(c * 512, (c + 1) * 512)
        ar = halves1.tile([P, CC, 512], BF16, tag="ar")
        nc.sync.dma_start(out=ar, in_=aown_dram.ap().rearrange("(cc p) t -> p cc t", p=P))
        s1 = halves.tile([P, CC, 512], FP32, tag="s1fo", name="s1")
        for cc in range(CC):
            nc.vector.tensor_scalar_add(out=s1[:, cc, :], in0=ar[:, cc, :],
                                        scalar1=bo_t[:, cc:cc + 1])
            nc.vector.tensor_add(out=s1[:, cc, :], in0=s1[:, cc, :],
                                 in1=x_own[:, cc, csl])
        layer_norm_chunk(s1, y32, yb, csl)

        # FFN
        for fc in range(FC):
            ps = psC.tile([P, 512], FP32, tag="psC")
            for cc in range(CC):
                nc.tensor.matmul(ps, w1_t[:, cc, fc * P:(fc + 1) * P], yb[:, cc, csl],
                                 start=(cc == 0), stop=(cc == CC - 1))
            nc.scalar.activation(out=h1T[:, fc, :], in_=ps, func=Relu,
                                 bias=b1_t[:, fc:fc + 1])
        fo = halves.tile([P, CC, 512], FP32, tag="s1fo", name="fo")
        for dc in range(CC):
            ps = psC.tile([P, 512], FP32, tag="psC")
            for fc in range(FC):
                nc.tensor.matmul(ps, w2_t[:, fc, dc * P:(dc + 1) * P], h1T[:, fc, :],
                                 start=(fc == 0), stop=(fc == FC - 1))
            nc.scalar.activation(out=fo[:, dc, :], in_=ps, func=Relu,
                                 bias=b2_t[:, dc:dc + 1])
            nc.vector.tensor_add(out=fo[:, dc, :], in0=fo[:, dc, :],
                                 in1=y32[:, dc, csl])

        # LN-B -> new x_own chunk (+ bf16 evict for AllGather unless last layer)
        if l < L - 1:
            xcb = tiny.tile([P, CC, 512], BF16, tag="xcb")
            layer_norm_chunk(fo, x_own, xcb, csl, slice(0, 512))
            key = "ab"[c]
            nc.sync.dma_start(
                out=D_[f"xh{key}"][l + 1].ap().rearrange("(cc p) t -> p cc t", p=P),
                in_=xcb)
            nc.gpsimd.collective_compute(
                kind="AllGather", op=mybir.AluOpType.bypass, replica_groups=GROUPS,
                ins=[D_[f"xh{key}"][l + 1].ap()], outs=[D_[f"xf{key}"][l + 1].ap()])
        else:
            layer_norm_chunk(fo, x_own, None, csl)

    # ---- load gathered-x chunk into xb (global order) ----
    def load_xb(l, key):
        xf = D_[f"xf{key}"][l]
        for s, tg in enumerate(CHUNK_TGS[key]):
            nc.sync.dma_start(
                out=xb[:, :, tg * 512:(tg + 1) * 512],
                in_=xf.ap()[s].rearrange("(cc p) t -> p cc t", p=P))

    # ---- embedding gather for own tokens -> x_own fp32 (+ AG chunks) ----
    for c in range(2):
        xcb = tiny.tile([P, CC, 512], BF16, tag="xcb")
        for tb in range(4):
            tcN = c * 4 + tb
            rows = small.tile([P, D], FP32, tag="emb_rows")
            nc.gpsimd.indirect_dma_start(
                out=rows, out_offset=None, in_=D_["emb"].ap(),
                in_offset=bass.IndirectOffsetOnAxis(ap=idx[:, tcN:tcN + 1], axis=0))
            for cc in range(CC):
                pt = psC.tile([P, 512], FP32, tag="psC", name="embT")
                nc.tensor.transpose(pt[:, 0:P], rows[:, cc * P:(cc + 1) * P], ident)
                nc.vector.tensor_copy(
                    out=x_own[:, cc, tcN * P:(tcN + 1) * P], in_=pt[:, 0:P])
                nc.scalar.copy(
                    out=xcb[:, cc, tb * P:(tb + 1) * P], in_=pt[:, 0:P])
        key = "ab"[c]
        nc.sync.dma_start(
            out=D_[f"xh{key}"][0].ap().rearrange("(cc p) t -> p cc t", p=P),
            in_=xcb)
        nc.gpsimd.collective_compute(
            kind="AllGather", op=mybir.AluOpType.bypass, replica_groups=GROUPS,
            ins=[D_[f"xh{key}"][0].ap()], outs=[D_[f"xf{key}"][0].ap()])

    for l in range(L):
        # ---- per-layer weights ----
        wq_t = wpool.tile([P, CC, DL], BF16, tag="wq")
        nc.sync.dma_start(out=wq_t, in_=D_["wq"].ap()[l].rearrange("(cc p) d -> p cc d", p=P))
        wk_t = wpool.tile([P, CC, DL], BF16, tag="wk")
        nc.sync.dma_start(out=wk_t, in_=D_["wk"].ap()[l].rearrange("(cc p) d -> p cc d", p=P))
        wv_t = wpool.tile([P, CC, DL], BF16, tag="wv")
        nc.sync.dma_start(out=wv_t, in_=D_["wv"].ap()[l].rearrange("(cc p) d -> p cc d", p=P))
        wo_t = wpool.tile([P, 2, D], BF16, tag="wo")
        nc.sync.dma_start(out=wo_t, in_=D_["wo"].ap()[l].rearrange("(hc p) d -> p hc d", p=P))
        bq_t = wpool.tile([P, 2], FP32, tag="bq")
        nc.sync.dma_start(out=bq_t, in_=D_["bq"].ap()[l].rearrange("(hc p) -> p hc", p=P))
        bk_t = wpool.tile([P, 2], FP32, tag="bk")
        nc.sync.dma_start(out=bk_t, in_=D_["bk"].ap()[l].rearrange("(hc p) -> p hc", p=P))
        bvB = wpool.tile([P, DL], FP32, tag="bvB")
        nc.sync.dma_start(out=bvB, in_=bass.AP(tensor=D_["bv"], offset=l * DL,
                                               ap=[[0, P], [1, DL]]))
        bo_t = wpool.tile([P, CC], FP32, tag="bo")
        nc.sync.dma_start(out=bo_t, in_=D_["bo"].ap()[l].rearrange("(cc p) -> p cc", p=P))
        b1_t = wpool.tile([P, FC], FP32, tag="b1")
        nc.sync.dma_start(out=b1_t, in_=D_["b1"].ap()[l].rearrange("(fc p) -> p fc", p=P))
        b2_t = wpool.tile([P, CC], FP32, tag="b2")
        nc.sync.dma_start(out=b2_t, in_=D_["b2"].ap()[l].rearrange("(cc p) -> p cc", p=P))
        w1_t = wbig.tile([P, CC, F], BF16, tag="w1")
        nc.sync.dma_start(out=w1_t, in_=D_["w1"].ap()[l].rearrange("(cc p) f -> p cc f", p=P))
        w2_t = wbig.tile([P, FC, D], BF16, tag="w2")
        nc.sync.dma_start(out=w2_t, in_=D_["w2"].ap()[l].rearrange("(fc p) d -> p fc d", p=P))

        # ---- QKV + attention, interleaved with RS chunks ----
        load_xb(l, "a")
        qkv_tg(0, wq_t, wk_t, wv_t, bq_t, bk_t, bvB)
        qkv_tg(2, wq_t, wk_t, wv_t, bq_t, bk_t, bvB)
        attn_qg(0, wo_t, D_["aparta"][l], 0)
        load_xb(l, "b")
        qkv_tg(1, wq_t, wk_t, wv_t, bq_t, bk_t, bvB)
        qkv_tg(3, wq_t, wk_t, wv_t, bq_t, bk_t, bvB)
        attn_qg(2, wo_t, D_["aparta"][l], 1)
        nc.gpsimd.collective_compute(
            kind="ReduceScatter", op=ADD, replica_groups=GROUPS,
            ins=[D_["aparta"][l].ap()], outs=[D_["aowna"][l].ap()])
        attn_qg(1, wo_t, D_["apartb"][l], 0)
        attn_qg(3, wo_t, D_["apartb"][l], 1)
        nc.gpsimd.collective_compute(
            kind="ReduceScatter", op=ADD, replica_groups=GROUPS,
            ins=[D_["apartb"][l].ap()], outs=[D_["aownb"][l].ap()])

        # ---- per-chunk residual + LN + FFN + LN (+ next-layer AG) ----
        mlp_chunk(l, 0, D_["aowna"][l], bo_t, b1_t, b2_t, w1_t, w2_t)
        mlp_chunk(l, 1, D_["aownb"][l], bo_t, b1_t, b2_t, w1_t, w2_t)

    # ---- output: transpose x_own back to rows [TOWN, D] ----
    for tb in range(TOWN // P):
        rows = small.tile([P, D], FP32, tag="emb_rows")
        for cc in range(CC):
            pt = psC.tile([P, 512], FP32, tag="psC", name="outT")
            nc.tensor.transpose(pt[:, 0:P], x_own[:, cc, tb * P:(tb + 1) * P], ident)
            nc.vector.tensor_copy(out=rows[:, cc * P:(cc + 1) * P], in_=pt[:, 0:P])
        nc.sync.dma_start(out=D_["out"].ap()[tb * P:(tb + 1) * P, :], in_=rows)

    ctx.close()


def _get_program():
    no_cc = bool(int(os.environ.get("BASS_ENC_NOCC", "0")))
    key = ("nc", no_cc)
    if key not in _CACHED:
        _CACHED[key] = _build_program(no_cc)
    return _CACHED[key]


def prep_in_maps(inputs):
    def f32(x):
        return np.ascontiguousarray(np.asarray(x, dtype=np.float32))

    def bf(x):
        return np.ascontiguousarray(np.asarray(x, dtype=np.float32).astype(ml_dtypes.bfloat16))

    source = np.asarray(inputs["source"]).astype(np.int32)
    emb = f32(inputs["emb"])
    ln_g, ln_b = f32(inputs["ln_g"]), f32(inputs["ln_b"])
    w1a, b1a = bf(inputs["w1"]), f32(inputs["b1"])
    w2a, b2a = bf(inputs["w2"]), f32(inputs["b2"])
    wqa, wka, wva = np.asarray(inputs["wq"]), np.asarray(inputs["wk"]), np.asarray(inputs["wv"])
    bqa, bka, bva = np.asarray(inputs["bq"]), np.asarray(inputs["bk"]), np.asarray(inputs["bv"])
    woa, boa = np.asarray(inputs["wo"]), f32(inputs["bo"])

    in_maps = []
    for core in range(8):
        b, half = core // 2, core % 2
        hsl = slice(half * DL, (half + 1) * DL)
        in_maps.append({
            "src": np.ascontiguousarray(source[b, half * TOWN:(half + 1) * TOWN]),
            "emb": emb,
            "wq": bf(wqa[:, :, hsl]), "wk": bf(wka[:, :, hsl]), "wv": bf(wva[:, :, hsl]),
            "bq": f32(bqa[:, hsl]), "bk": f32(bka[:, hsl]), "bv": f32(bva[:, hsl]),
            "wo": bf(woa[:, hsl, :]), "bo": boa,
            "w1": w1a, "b1": b1a, "w2": w2a, "b2": b2a,
            "ln_g": ln_g, "ln_b": ln_b,
        })
    return in_maps


def kernel(**inputs):
    nc = _get_program()
    in_maps = prep_in_maps(inputs)
    trace = bool(int(os.environ.get("BASS_ENC_TRACE", "0")))
    res = bass_utils.run_bass_kernel_spmd(nc, in_maps, core_ids=list(range(8)),
                                          trace=trace)
    _CACHED["last_results"] = res

    outp = np.empty((B, S, D), np.float32)
    for core in range(8):
        b, half = core // 2, core % 2
        outp[b, half * TOWN:(half + 1) * TOWN, :] = res.results[core]["out"]
    return outp
